# revision 22
# baseline (speedup 1.0000x reference)
"""Trainium2 Bass kernel for nn_ConvTwist (twisted grouped conv).

Problem: x (32, 512, 56, 56) f32, W (512, 8, 3, 3) f32.
The full 512x512x3x3 kernel is block-diagonal over 16 independent 32-channel
blocks (the group-twist permutation j(i) = i+3 if i%4==0 else i-1 stays inside
blocks of 4 groups = 32 channels). Each block is a dense 32->32 3x3 conv
(with 4 nonzero 8x8 group sub-blocks).

Strategy (per core, data-parallel over batch, 4 images/core):
- Host pre-permutes channels, pads rows to 58 cols, casts to fp16, and builds
  per-tile 32x32 lhsT weight matrices for the 9 kernel offsets.
- Device: conv = 9 shifted matmuls accumulated in PSUM. The PE array is split
  into 16 concurrent 32x32 tiles (tile_position); tile (i,j) handles channel
  block 4i+j: rhs from SBUF partitions 32i (region j), output to PSUM bank i
  partitions 32j. Output channel c = 128*bank + partition, so results land in
  natural channel order for a strided store.
- 56 rows are processed in 7 bands of 8 rows (N=448 <= 512 PSUM bank limit).
- Weights are loaded once per (offset, tile) per band-pair via explicit
  LDWEIGHTS; the matmuls are non-self-loading (ldweights=False) so the PE
  does not reload the stationary operand for every matmul.
- PSUM is evacuated with f32->f16 casting copies split across the Vector and
  Scalar engines; the fp16 output is DMAed per band-group and upcast on host.
"""
import numpy as np

import concourse.bacc as bacc
import concourse.mybir as mybir
import concourse.tile as tile
from concourse.tile import add_dep_helper
from concourse.bass_utils import run_bass_kernel_spmd


def _dedupe_ldweights(nc):
    """Remove InstLdweights that reload the exact weights already resident in
    their PE tile (the tile legalizer emits one load per matmul; the two
    band-matmuls of a pair share the same stationary operand)."""
    for blk in nc.main_func.blocks:
        insts = blk.instructions
        last = {}  # tile_position -> weights signature
        dead = []
        for pos, i in enumerate(insts):
            if isinstance(i, mybir.InstLdweights):
                tp = tuple(i.tile_position) if i.tile_position else (0, 0)
                sig = str(i.ins[0])
                si = i.sync_info
                clean = si is None or (not si.on_wait and not si.on_update)
                if last.get(tp) == sig and clean:
                    dead.append(i)
                else:
                    last[tp] = sig
        for i in dead:
            insts.remove(i)
    return nc


def _prune_mm_sem_incs(nc):
    """The tile framework makes every matmul then_inc the PE semaphore; the
    increments serialize (~26ns each) and throttle the matmul stream. Only
    increments actually referenced by a wait are needed (matmuls complete in
    pc order, so any wait `sem >= v` is equivalent to completion of the v-th
    incrementing matmul). Keep exactly the referenced increments and renumber
    the waits."""
    all_insts = [i for blk in nc.main_func.blocks for i in blk.instructions]
    upd = {}  # sem id -> [instr in program order]
    for i in all_insts:
        if isinstance(i, mybir.InstMatmult):
            si = i.sync_info
            if si:
                for u in si.on_update:
                    if u.update_mode == "sem-inc" and u.update_value == 1:
                        upd.setdefault(u.id, []).append(i)
    for sem_id, updaters in upd.items():
        # every reference to this sem must be an inc-by-1 from a matmul or a
        # ge-imm wait, otherwise leave the whole semaphore untouched
        waits = []
        safe = True
        for i in all_insts:
            si = getattr(i, "sync_info", None)
            if si:
                for w in si.on_wait:
                    if w.id == sem_id:
                        if w.wait_mode != "sem-ge-imm":
                            safe = False
                        waits.append(w)
                for u in si.on_update:
                    if u.id == sem_id and not (
                            isinstance(i, mybir.InstMatmult)
                            and u.update_mode == "sem-inc"
                            and u.update_value == 1):
                        safe = False
        if not waits or not safe:
            continue
        if True:
            keep = sorted({w.wait_value - 1 for w in waits
                           if 1 <= w.wait_value <= len(updaters)})
            rank = {}
            for r, k in enumerate(keep):
                rank[k] = r + 1
            for w in waits:
                if 1 <= w.wait_value <= len(updaters):
                    w.wait_value = rank[w.wait_value - 1]
            keep_set = {id(updaters[k]) for k in keep}
            for inst in updaters:
                if id(inst) not in keep_set:
                    si = inst.sync_info
                    si.on_update = [u for u in si.on_update if u.id != sem_id]
    return nc

N_CORES = 8
B = 32               # full batch
BC = B // N_CORES    # images per core
C = 512              # channels
H = W_ = 56          # spatial
WP = 58              # padded row width
HWP = H * WP         # 3248 padded pixels / channel
HW = H * W_          # 3136 pixels / channel
BAND = 8             # rows per band
NB = H // BAND       # 7 bands
NBAND = BAND * W_    # 448 free-dim per band
GROUPS = [(0, 1), (2, 3), (4, 5), (6,)]

# offset order: dy=0 first so the start=True matmul covers the full band
OFFS = [(0, -1), (0, 0), (0, 1), (-1, -1), (-1, 0), (-1, 1), (1, -1), (1, 0), (1, 1)]

F16 = mybir.dt.float16
F32 = mybir.dt.float32

_CACHE = {}


def _build_nc():
    nc = bacc.Bacc(None, target_bir_lowering=False)
    x_d = nc.dram_tensor("x", [BC, 128, 4 * HWP], F16, kind="ExternalInput")
    w_d = nc.dram_tensor("w", [128, 4 * 9 * 32], F16, kind="ExternalInput")
    o_d = nc.dram_tensor("o", [BC, C, HW], F16, kind="ExternalOutput")

    with tile.TileContext(nc) as tc:
        with (
            tc.tile_pool(name="xp", bufs=2) as xpool,
            tc.tile_pool(name="wp", bufs=1) as wpool,
            tc.tile_pool(name="op", bufs=2) as opool,
            tc.tile_pool(name="ps", bufs=2, space="PSUM") as pspool,
        ):
            wt = wpool.tile([128, 4 * 9 * 32], F16, tag="w", name="wt")
            nc.sync.dma_start(out=wt[:], in_=w_d[:])
            for n in range(BC):
                xt = xpool.tile([128, 4 * HWP], F16, tag="x", name=f"xt{n}")
                # split the load so early bands' matmuls can start sooner
                # (subtile deps let band b wait only on the rows it reads)
                xr = x_d[n].rearrange("p (r f) -> p r f", r=4)
                xtr = xt.rearrange("p (r f) -> p r f", r=4)
                cuts = [0, 15 * WP, 30 * WP, 45 * WP, HWP]
                for ci in range(4):
                    nc.sync.dma_start(out=xtr[:, :, cuts[ci]:cuts[ci + 1]],
                                      in_=xr[:, :, cuts[ci]:cuts[ci + 1]])
                # per-channel padded image view: [part, region, row, col]
                xv = xt.rearrange("p (r y c) -> p r y c", r=4, c=WP)
                # the version suffix keeps the BIR hash distinct from earlier
                # no-ldw-opt builds so the compile cache can't serve a stale NEFF
                ot = opool.tile([128, 4 * HW], F16, tag="o", name=f"ot{n}_v9")
                otv = ot.rearrange("p (i f) -> p i f", i=4)
                for bands in GROUPS:
                    # one 4-bank PSUM tile per band; weights loaded once per
                    # (offset, tile) serve both bands of the group.
                    pst = {b: pspool.tile([128, 4 * 512], F32, tag=f"ps{b % 2}",
                                          name=f"ps{n}_{b}", bufs=1)
                           for b in bands}
                    # diagonal tile order: adjacent instructions hit
                    # different PE row groups and column groups, so the
                    # weight load for one tile can overlap in-flight
                    # matmuls of the neighbouring tiles.
                    diag = [((t % 4), (t + t // 4) % 4) for t in range(16)]

                    def mm(i, j, b, o_idx, dy, dx):
                        lhsT = wt[32 * i:32 * i + 32,
                                  (j * 9 + o_idx) * 32:(j * 9 + o_idx) * 32 + 32]
                        r0 = b * BAND
                        y0 = max(r0, -dy)
                        y1 = min(r0 + BAND, 56 - max(0, dy))
                        nr = y1 - y0
                        po = (y0 - r0) * W_
                        rhs = xv[32 * i:32 * i + 32, j,
                                 y0 + dy:y0 + dy + nr, 1 + dx:1 + dx + W_]
                        nc.tensor.matmul(
                            pst[b][32 * j:32 * j + 32,
                                   512 * i + po:512 * i + po + nr * W_],
                            lhsT, rhs,
                            start=(o_idx == 0), stop=(o_idx == len(OFFS) - 1),
                            tile_position=(32 * i, 32 * j))

                    for o_idx, (dy, dx) in enumerate(OFFS):
                        if o_idx in (0, len(OFFS) - 1) and len(bands) == 2:
                            # de-phase the two bands at the group edges: the
                            # first wave's even-band matmuls only need the
                            # PSUM banks the previous group released first,
                            # and the last wave finishes the even band early
                            # so its evacuation overlaps the odd band's tail.
                            # (the ldweights dedupe keeps one load per tile)
                            for i, j in diag:
                                mm(i, j, bands[0], o_idx, dy, dx)
                            for i, j in diag:
                                mm(i, j, bands[1], o_idx, dy, dx)
                        else:
                            for i, j in diag:
                                for b in bands:
                                    mm(i, j, b, o_idx, dy, dx)
                    # evacuation: vector takes banks 0-1, scalar banks 2-3,
                    # even band first on both engines so its banks free first
                    for b in bands:
                        pv = pst[b].rearrange("p (i f) -> p i f", i=4)
                        nc.vector.tensor_copy(
                            out=otv[:, 0:2, b * NBAND:(b + 1) * NBAND],
                            in_=pv[:, 0:2, 0:NBAND])
                        nc.scalar.copy(
                            out=otv[:, 2:4, b * NBAND:(b + 1) * NBAND],
                            in_=pv[:, 2:4, 0:NBAND])
                    f0 = bands[0] * NBAND
                    f1 = (bands[-1] + 1) * NBAND
                    dst = o_d[n].rearrange("(i p) f -> p i f", p=128)[:, :, f0:f1]
                    nc.sync.dma_start(out=dst, in_=otv[:, :, f0:f1])
    _dedupe_ldweights(nc)
    _prune_mm_sem_incs(nc)
    nc.compile()
    return nc


def _prep_weights(W: np.ndarray) -> np.ndarray:
    """W (512, 8, 3, 3) f32 -> (128, 4*9*32) f16 lhsT layout.

    partition p = 32*i + k ; free idx = (j*9 + o)*32 + m
    holds W_blk[4i+j][m, k, dy, dx] for offset o = OFFS[o_idx].
    """
    Wg = W.reshape(64, 8, 8, 3, 3)  # [group gi][oc][ic][dy][dx]
    # block-level dense 32x32 kernels
    Wb = np.zeros((16, 32, 32, 3, 3), dtype=np.float32)  # [b][m(out)][k(in)][dy][dx]
    for gi in range(64):
        b, u = divmod(gi, 4)
        jg = gi + 3 if gi % 4 == 0 else gi - 1  # input group (twist)
        v = jg % 4
        assert jg // 4 == b
        Wb[b, 8 * u:8 * u + 8, 8 * v:8 * v + 8] = Wg[gi]
    out = np.zeros((128, 4 * 9 * 32), dtype=np.float32)
    for i in range(4):
        for j in range(4):
            blk = Wb[4 * i + j]  # [m][k][dy][dx]
            for o_idx, (dy, dx) in enumerate(OFFS):
                # lhsT[k, m]
                out[32 * i:32 * i + 32, (j * 9 + o_idx) * 32:(j * 9 + o_idx) * 32 + 32] = \
                    blk[:, :, dy + 1, dx + 1].T
    return out.astype(np.float16)


def _prep_x(x_shard: np.ndarray) -> np.ndarray:
    """x_shard (BC, 512, 56, 56) f32 -> (BC, 128, 4*HWP) f16 permuted+padded.

    Device partition p = 32*s + k of region r holds original channel
    c = 128*s + 32*r + k (so tile (i,j) reading region j, slice i gets
    block 4i+j), padded to 58 cols.
    """
    n = x_shard.shape[0]
    xs = x_shard.reshape(n, 4, 4, 32, H, W_)          # [n][s][r][k][y][x]
    xs = xs.transpose(0, 1, 3, 2, 4, 5)               # [n][s][k][r][y][x]
    xp = np.zeros((n, 4, 32, 4, H, WP), dtype=np.float16)
    xp[..., 1:57] = xs
    return xp.reshape(n, 128, 4, HWP).reshape(n, 128, 4 * HWP)


def kernel(x: np.ndarray, W: np.ndarray) -> np.ndarray:
    if "nc" not in _CACHE:
        _CACHE["nc"] = _build_nc()
    nc = _CACHE["nc"]

    w_dev = _prep_weights(np.asarray(W, dtype=np.float32))
    x = np.asarray(x, dtype=np.float32)
    in_maps = []
    for c in range(N_CORES):
        shard = x[c * BC:(c + 1) * BC]
        in_maps.append({"x": _prep_x(shard), "w": w_dev})

    res = run_bass_kernel_spmd(nc, in_maps, core_ids=list(range(N_CORES)))
    outs = [res.results[c]["o"].reshape(BC, C, H, W_).astype(np.float32)
            for c in range(N_CORES)]
    return np.concatenate(outs, axis=0)


if __name__ == "__main__":
    # quick self-test against a numpy reference
    rng = np.random.default_rng(0)
    x = rng.standard_normal((B, C, H, W_), dtype=np.float32)
    Wt = (rng.standard_normal((C, 8, 3, 3)) * 0.12).astype(np.float32)
    out = kernel(x, Wt)
    print("out", out.shape, out.dtype)


# revision 23
# speedup vs baseline: 1.0733x; 1.0733x over previous
"""Trainium2 Bass kernel for nn_ConvTwist (twisted grouped conv).

Problem: x (32, 512, 56, 56) f32, W (512, 8, 3, 3) f32.
The full 512x512x3x3 kernel is block-diagonal over 16 independent 32-channel
blocks (the group-twist permutation j(i) = i+3 if i%4==0 else i-1 stays inside
blocks of 4 groups = 32 channels). Each block is a dense 32->32 3x3 conv
(with 4 nonzero 8x8 group sub-blocks).

Strategy (per core, data-parallel over batch, 4 images/core):
- Host pre-permutes channels, pads rows to 58 cols, casts to fp16, and builds
  per-tile 32x32 lhsT weight matrices for the 9 kernel offsets.
- Device: conv = 9 shifted matmuls accumulated in PSUM. The PE array is split
  into 16 concurrent 32x32 tiles (tile_position); tile (i,j) handles channel
  block 4i+j: rhs from SBUF partitions 32i (region j), output to PSUM bank i
  partitions 32j. Output channel c = 128*bank + partition, so results land in
  natural channel order for a strided store.
- 56 rows are processed in 7 bands of 8 rows (N=448 <= 512 PSUM bank limit).
- Weights are loaded once per (offset, tile) per band-pair via explicit
  LDWEIGHTS; the matmuls are non-self-loading (ldweights=False) so the PE
  does not reload the stationary operand for every matmul.
- PSUM is evacuated with f32->f16 casting copies split across the Vector and
  Scalar engines; the fp16 output is DMAed per band-group and upcast on host.
"""
import numpy as np

import concourse.bacc as bacc
import concourse.mybir as mybir
import concourse.tile as tile
from concourse.tile import add_dep_helper
from concourse.bass_utils import run_bass_kernel_spmd


def _dedupe_ldweights(nc):
    """Remove InstLdweights that reload the exact weights already resident in
    their PE tile (the tile legalizer emits one load per matmul; the two
    band-matmuls of a pair share the same stationary operand)."""
    for blk in nc.main_func.blocks:
        insts = blk.instructions
        last = {}  # tile_position -> weights signature
        dead = []
        for pos, i in enumerate(insts):
            if isinstance(i, mybir.InstLdweights):
                tp = tuple(i.tile_position) if i.tile_position else (0, 0)
                sig = str(i.ins[0])
                si = i.sync_info
                clean = si is None or (not si.on_wait and not si.on_update)
                if last.get(tp) == sig and clean:
                    dead.append(i)
                else:
                    last[tp] = sig
        for i in dead:
            insts.remove(i)
    return nc


def _prune_mm_sem_incs(nc):
    """The tile framework makes every matmul then_inc the PE semaphore; the
    increments serialize (~26ns each) and throttle the matmul stream. Only
    increments actually referenced by a wait are needed (matmuls complete in
    pc order, so any wait `sem >= v` is equivalent to completion of the v-th
    incrementing matmul). Keep exactly the referenced increments and renumber
    the waits."""
    all_insts = [i for blk in nc.main_func.blocks for i in blk.instructions]
    upd = {}  # sem id -> [instr in program order]
    for i in all_insts:
        if isinstance(i, mybir.InstMatmult):
            si = i.sync_info
            if si:
                for u in si.on_update:
                    if u.update_mode == "sem-inc" and u.update_value == 1:
                        upd.setdefault(u.id, []).append(i)
    for sem_id, updaters in upd.items():
        # every reference to this sem must be an inc-by-1 from a matmul or a
        # ge-imm wait, otherwise leave the whole semaphore untouched
        waits = []
        safe = True
        for i in all_insts:
            si = getattr(i, "sync_info", None)
            if si:
                for w in si.on_wait:
                    if w.id == sem_id:
                        if w.wait_mode != "sem-ge-imm":
                            safe = False
                        waits.append(w)
                for u in si.on_update:
                    if u.id == sem_id and not (
                            isinstance(i, mybir.InstMatmult)
                            and u.update_mode == "sem-inc"
                            and u.update_value == 1):
                        safe = False
        if not waits or not safe:
            continue
        if True:
            keep = sorted({w.wait_value - 1 for w in waits
                           if 1 <= w.wait_value <= len(updaters)})
            rank = {}
            for r, k in enumerate(keep):
                rank[k] = r + 1
            for w in waits:
                if 1 <= w.wait_value <= len(updaters):
                    w.wait_value = rank[w.wait_value - 1]
            keep_set = {id(updaters[k]) for k in keep}
            for inst in updaters:
                if id(inst) not in keep_set:
                    si = inst.sync_info
                    si.on_update = [u for u in si.on_update if u.id != sem_id]
    return nc

N_CORES = 8
B = 32               # full batch
BC = B // N_CORES    # images per core
C = 512              # channels
H = W_ = 56          # spatial
WP = 58              # padded row width
HWP = H * WP         # 3248 padded pixels / channel
HW = H * W_          # 3136 pixels / channel
BAND = 8             # rows per band
NB = H // BAND       # 7 bands
NBAND = BAND * W_    # 448 free-dim per band
GROUPS = [(0, 1), (2, 3), (4, 5), (6,)]

# offset order: dy=0 first so the start=True matmul covers the full band
OFFS = [(0, -1), (0, 0), (0, 1), (-1, -1), (-1, 0), (-1, 1), (1, -1), (1, 0), (1, 1)]

F16 = mybir.dt.float16
F32 = mybir.dt.float32

_CACHE = {}


def _build_nc():
    nc = bacc.Bacc(None, target_bir_lowering=False)
    x_d = nc.dram_tensor("x", [BC, 128, 4 * HWP], F16, kind="ExternalInput")
    w_d = nc.dram_tensor("w", [128, 4 * 9 * 32], F16, kind="ExternalInput")
    o_d = nc.dram_tensor("o", [BC, C, HW], F16, kind="ExternalOutput")

    with tile.TileContext(nc) as tc:
        with (
            tc.tile_pool(name="xp", bufs=2) as xpool,
            tc.tile_pool(name="wp", bufs=1) as wpool,
            tc.tile_pool(name="op", bufs=2) as opool,
            tc.tile_pool(name="ps", bufs=2, space="PSUM") as pspool,
        ):
            wt = wpool.tile([128, 4 * 9 * 32], F16, tag="w", name="wt")
            nc.sync.dma_start(out=wt[:], in_=w_d[:])
            for n in range(BC):
                xt = xpool.tile([128, 4 * HWP], F16, tag="x", name=f"xt{n}")
                # split the load so early bands' matmuls can start sooner
                # (subtile deps let band b wait only on the rows it reads)
                xr = x_d[n].rearrange("p (r f) -> p r f", r=4)
                xtr = xt.rearrange("p (r f) -> p r f", r=4)
                cuts = [0, 15 * WP, 30 * WP, 45 * WP, HWP]
                for ci in range(4):
                    nc.sync.dma_start(out=xtr[:, :, cuts[ci]:cuts[ci + 1]],
                                      in_=xr[:, :, cuts[ci]:cuts[ci + 1]])
                # per-channel padded image view: [part, region, row, col]
                xv = xt.rearrange("p (r y c) -> p r y c", r=4, c=WP)
                # the version suffix keeps the BIR hash distinct from earlier
                # no-ldw-opt builds so the compile cache can't serve a stale NEFF
                ot = opool.tile([128, 4 * HW], F16, tag="o", name=f"ot{n}_v10")
                otv = ot.rearrange("p (i f) -> p i f", i=4)
                for bands in GROUPS:
                    # one 4-bank PSUM tile per band; weights loaded once per
                    # (offset, tile) serve both bands of the group.
                    pst = {b: pspool.tile([128, 4 * 512], F32, tag=f"ps{b % 2}",
                                          name=f"ps{n}_{b}", bufs=1)
                           for b in bands}
                    # diagonal tile order: adjacent instructions hit
                    # different PE row groups and column groups, so the
                    # weight load for one tile can overlap in-flight
                    # matmuls of the neighbouring tiles.
                    diag = [((t % 4), (t + t // 4) % 4) for t in range(16)]

                    def mm(i, j, b, o_idx, dy, dx):
                        lhsT = wt[32 * i:32 * i + 32,
                                  (j * 9 + o_idx) * 32:(j * 9 + o_idx) * 32 + 32]
                        r0 = b * BAND
                        y0 = max(r0, -dy)
                        y1 = min(r0 + BAND, 56 - max(0, dy))
                        nr = y1 - y0
                        po = (y0 - r0) * W_
                        rhs = xv[32 * i:32 * i + 32, j,
                                 y0 + dy:y0 + dy + nr, 1 + dx:1 + dx + W_]
                        nc.tensor.matmul(
                            pst[b][32 * j:32 * j + 32,
                                   512 * i + po:512 * i + po + nr * W_],
                            lhsT, rhs,
                            start=(o_idx == 0), stop=(o_idx == len(OFFS) - 1),
                            tile_position=(32 * i, 32 * j))

                    for o_idx, (dy, dx) in enumerate(OFFS):
                        for i, j in diag:
                            for b in bands:
                                mm(i, j, b, o_idx, dy, dx)
                    # evacuation: vector takes banks 0-1, scalar banks 2-3,
                    # even band first on both engines so its banks free first
                    for b in bands:
                        pv = pst[b].rearrange("p (i f) -> p i f", i=4)
                        nc.vector.tensor_copy(
                            out=otv[:, 0:2, b * NBAND:(b + 1) * NBAND],
                            in_=pv[:, 0:2, 0:NBAND])
                        nc.scalar.copy(
                            out=otv[:, 2:4, b * NBAND:(b + 1) * NBAND],
                            in_=pv[:, 2:4, 0:NBAND])
                    f0 = bands[0] * NBAND
                    f1 = (bands[-1] + 1) * NBAND
                    dst = o_d[n].rearrange("(i p) f -> p i f", p=128)[:, :, f0:f1]
                    nc.sync.dma_start(out=dst, in_=otv[:, :, f0:f1])
    _dedupe_ldweights(nc)
    _prune_mm_sem_incs(nc)
    nc.compile()
    return nc


def _prep_weights(W: np.ndarray) -> np.ndarray:
    """W (512, 8, 3, 3) f32 -> (128, 4*9*32) f16 lhsT layout.

    partition p = 32*i + k ; free idx = (j*9 + o)*32 + m
    holds W_blk[4i+j][m, k, dy, dx] for offset o = OFFS[o_idx].
    """
    Wg = W.reshape(64, 8, 8, 3, 3)  # [group gi][oc][ic][dy][dx]
    # block-level dense 32x32 kernels
    Wb = np.zeros((16, 32, 32, 3, 3), dtype=np.float32)  # [b][m(out)][k(in)][dy][dx]
    for gi in range(64):
        b, u = divmod(gi, 4)
        jg = gi + 3 if gi % 4 == 0 else gi - 1  # input group (twist)
        v = jg % 4
        assert jg // 4 == b
        Wb[b, 8 * u:8 * u + 8, 8 * v:8 * v + 8] = Wg[gi]
    out = np.zeros((128, 4 * 9 * 32), dtype=np.float32)
    for i in range(4):
        for j in range(4):
            blk = Wb[4 * i + j]  # [m][k][dy][dx]
            for o_idx, (dy, dx) in enumerate(OFFS):
                # lhsT[k, m]
                out[32 * i:32 * i + 32, (j * 9 + o_idx) * 32:(j * 9 + o_idx) * 32 + 32] = \
                    blk[:, :, dy + 1, dx + 1].T
    return out.astype(np.float16)


def _prep_x(x_shard: np.ndarray) -> np.ndarray:
    """x_shard (BC, 512, 56, 56) f32 -> (BC, 128, 4*HWP) f16 permuted+padded.

    Device partition p = 32*s + k of region r holds original channel
    c = 128*s + 32*r + k (so tile (i,j) reading region j, slice i gets
    block 4i+j), padded to 58 cols.
    """
    n = x_shard.shape[0]
    xs = x_shard.reshape(n, 4, 4, 32, H, W_)          # [n][s][r][k][y][x]
    xs = xs.transpose(0, 1, 3, 2, 4, 5)               # [n][s][k][r][y][x]
    xp = np.zeros((n, 4, 32, 4, H, WP), dtype=np.float16)
    xp[..., 1:57] = xs
    return xp.reshape(n, 128, 4, HWP).reshape(n, 128, 4 * HWP)


def kernel(x: np.ndarray, W: np.ndarray) -> np.ndarray:
    if "nc" not in _CACHE:
        _CACHE["nc"] = _build_nc()
    nc = _CACHE["nc"]

    w_dev = _prep_weights(np.asarray(W, dtype=np.float32))
    x = np.asarray(x, dtype=np.float32)
    in_maps = []
    for c in range(N_CORES):
        shard = x[c * BC:(c + 1) * BC]
        in_maps.append({"x": _prep_x(shard), "w": w_dev})

    res = run_bass_kernel_spmd(nc, in_maps, core_ids=list(range(N_CORES)))
    outs = [res.results[c]["o"].reshape(BC, C, H, W_).astype(np.float32)
            for c in range(N_CORES)]
    return np.concatenate(outs, axis=0)


if __name__ == "__main__":
    # quick self-test against a numpy reference
    rng = np.random.default_rng(0)
    x = rng.standard_normal((B, C, H, W_), dtype=np.float32)
    Wt = (rng.standard_normal((C, 8, 3, 3)) * 0.12).astype(np.float32)
    out = kernel(x, Wt)
    print("out", out.shape, out.dtype)


# revision 28
# speedup vs baseline: 1.1113x; 1.0355x over previous
"""Trainium2 Bass kernel for nn_ConvTwist (twisted grouped conv).

Problem: x (32, 512, 56, 56) f32, W (512, 8, 3, 3) f32.
The full 512x512x3x3 kernel is block-diagonal over 16 independent 32-channel
blocks (the group-twist permutation j(i) = i+3 if i%4==0 else i-1 stays inside
blocks of 4 groups = 32 channels). Each block is a dense 32->32 3x3 conv
(with 4 nonzero 8x8 group sub-blocks).

Strategy (per core, data-parallel over batch, 4 images/core):
- Host pre-permutes channels, pads rows to 58 cols, casts to fp16, and builds
  per-tile 32x32 lhsT weight matrices for the 9 kernel offsets.
- Device: conv = 9 shifted matmuls accumulated in PSUM. The PE array is split
  into 16 concurrent 32x32 tiles (tile_position); tile (i,j) handles channel
  block 4i+j: rhs from SBUF partitions 32i (region j), output to PSUM bank i
  partitions 32j. Output channel c = 128*bank + partition, so results land in
  natural channel order for a strided store.
- 56 rows are processed in 7 bands of 8 rows (N=448 <= 512 PSUM bank limit).
- Weights are loaded once per (offset, tile) per band-pair via explicit
  LDWEIGHTS; the matmuls are non-self-loading (ldweights=False) so the PE
  does not reload the stationary operand for every matmul.
- PSUM is evacuated with f32->f16 casting copies split across the Vector and
  Scalar engines; the fp16 output is DMAed per band-group and upcast on host.
"""
import numpy as np

import concourse.bacc as bacc
import concourse.mybir as mybir
import concourse.tile as tile
from concourse.tile import add_dep_helper
from concourse.bass_utils import run_bass_kernel_spmd


def _dedupe_ldweights(nc):
    """Remove InstLdweights that reload the exact weights already resident in
    their PE tile (the tile legalizer emits one load per matmul; the two
    band-matmuls of a pair share the same stationary operand)."""
    for blk in nc.main_func.blocks:
        insts = blk.instructions
        last = {}  # tile_position -> weights signature
        dead = []
        for pos, i in enumerate(insts):
            if isinstance(i, mybir.InstLdweights):
                tp = tuple(i.tile_position) if i.tile_position else (0, 0)
                sig = str(i.ins[0])
                si = i.sync_info
                clean = si is None or (not si.on_wait and not si.on_update)
                if last.get(tp) == sig and clean:
                    dead.append(i)
                else:
                    last[tp] = sig
        for i in dead:
            insts.remove(i)
    return nc


def _prune_mm_sem_incs(nc):
    """The tile framework makes every matmul then_inc the PE semaphore; the
    increments serialize (~26ns each) and throttle the matmul stream. Only
    increments actually referenced by a wait are needed (matmuls complete in
    pc order, so any wait `sem >= v` is equivalent to completion of the v-th
    incrementing matmul). Keep exactly the referenced increments and renumber
    the waits."""
    all_insts = [i for blk in nc.main_func.blocks for i in blk.instructions]
    upd = {}  # sem id -> [instr in program order]
    for i in all_insts:
        if isinstance(i, mybir.InstMatmult):
            si = i.sync_info
            if si:
                for u in si.on_update:
                    if u.update_mode == "sem-inc" and u.update_value == 1:
                        upd.setdefault(u.id, []).append(i)
    for sem_id, updaters in upd.items():
        # every reference to this sem must be an inc-by-1 from a matmul or a
        # ge-imm wait, otherwise leave the whole semaphore untouched
        waits = []
        safe = True
        for i in all_insts:
            si = getattr(i, "sync_info", None)
            if si:
                for w in si.on_wait:
                    if w.id == sem_id:
                        if w.wait_mode != "sem-ge-imm":
                            safe = False
                        waits.append(w)
                for u in si.on_update:
                    if u.id == sem_id and not (
                            isinstance(i, mybir.InstMatmult)
                            and u.update_mode == "sem-inc"
                            and u.update_value == 1):
                        safe = False
        if not waits or not safe:
            continue
        if True:
            keep = sorted({w.wait_value - 1 for w in waits
                           if 1 <= w.wait_value <= len(updaters)})
            rank = {}
            for r, k in enumerate(keep):
                rank[k] = r + 1
            for w in waits:
                if 1 <= w.wait_value <= len(updaters):
                    w.wait_value = rank[w.wait_value - 1]
            keep_set = {id(updaters[k]) for k in keep}
            for inst in updaters:
                if id(inst) not in keep_set:
                    si = inst.sync_info
                    si.on_update = [u for u in si.on_update if u.id != sem_id]
    return nc

N_CORES = 8
B = 32               # full batch
BC = B // N_CORES    # images per core
C = 512              # channels
H = W_ = 56          # spatial
WP = 58              # padded row width
HWP = H * WP         # 3248 padded pixels / channel
HW = H * W_          # 3136 pixels / channel
BAND = 8             # rows per band
NB = H // BAND       # 7 bands
NBAND = BAND * W_    # 448 free-dim per band
GROUPS = [(0, 1), (2, 3), (4, 5), (6,)]

# offset order: dy=0 first so the start=True matmul covers the full band
OFFS = [(0, -1), (0, 0), (0, 1), (-1, -1), (-1, 0), (-1, 1), (1, -1), (1, 0), (1, 1)]

F16 = mybir.dt.float16
F32 = mybir.dt.float32

_CACHE = {}


def _build_nc():
    nc = bacc.Bacc(None, target_bir_lowering=False)
    x_d = nc.dram_tensor("x", [BC, 128, 4 * HWP], F16, kind="ExternalInput")
    w_d = nc.dram_tensor("w", [128, 4 * 9 * 32], F16, kind="ExternalInput")
    o_d = nc.dram_tensor("o", [BC, C, HW], F16, kind="ExternalOutput")

    with tile.TileContext(nc) as tc:
        with (
            tc.tile_pool(name="xp", bufs=2) as xpool,
            tc.tile_pool(name="wp", bufs=1) as wpool,
            tc.tile_pool(name="op", bufs=2) as opool,
            tc.tile_pool(name="ps", bufs=2, space="PSUM") as pspool,
        ):
            wt = wpool.tile([128, 4 * 9 * 32], F16, tag="w", name="wt")
            nc.sync.dma_start(out=wt[:], in_=w_d[:])
            for n in range(BC):
                xt = xpool.tile([128, 4 * HWP], F16, tag="x", name=f"xt{n}")
                # split the load so early bands' matmuls can start sooner
                # (subtile deps let band b wait only on the rows it reads)
                xr = x_d[n].rearrange("p (r f) -> p r f", r=4)
                xtr = xt.rearrange("p (r f) -> p r f", r=4)
                # chunk boundaries aligned to what each band group reads
                # (group g needs rows < cuts[g+2]); the tiny first chunk lets
                # band 0 start as soon as possible
                cuts = [0, 9 * WP, 17 * WP, 33 * WP, 49 * WP, HWP]
                for ci in range(5):
                    nc.sync.dma_start(out=xtr[:, :, cuts[ci]:cuts[ci + 1]],
                                      in_=xr[:, :, cuts[ci]:cuts[ci + 1]])
                # per-channel padded image view: [part, region, row, col]
                xv = xt.rearrange("p (r y c) -> p r y c", r=4, c=WP)
                # the version suffix keeps the BIR hash distinct from earlier
                # no-ldw-opt builds so the compile cache can't serve a stale NEFF
                ot = opool.tile([128, 4 * HW], F16, tag="o", name=f"ot{n}_v12")
                otv = ot.rearrange("p (i f) -> p i f", i=4)
                for bands in GROUPS:
                    # one 4-bank PSUM tile per band; weights loaded once per
                    # (offset, tile) serve both bands of the group.
                    pst = {b: pspool.tile([128, 4 * 512], F32, tag=f"ps{b % 2}",
                                          name=f"ps{n}_{b}", bufs=1)
                           for b in bands}
                    # diagonal tile order: adjacent instructions hit
                    # different PE row groups and column groups, so the
                    # weight load for one tile can overlap in-flight
                    # matmuls of the neighbouring tiles.
                    diag = [((t % 4), (t + t // 4) % 4) for t in range(16)]

                    def mm(i, j, b, o_idx, dy, dx):
                        lhsT = wt[32 * i:32 * i + 32,
                                  (j * 9 + o_idx) * 32:(j * 9 + o_idx) * 32 + 32]
                        r0 = b * BAND
                        y0 = max(r0, -dy)
                        y1 = min(r0 + BAND, 56 - max(0, dy))
                        nr = y1 - y0
                        po = (y0 - r0) * W_
                        rhs = xv[32 * i:32 * i + 32, j,
                                 y0 + dy:y0 + dy + nr, 1 + dx:1 + dx + W_]
                        nc.tensor.matmul(
                            pst[b][32 * j:32 * j + 32,
                                   512 * i + po:512 * i + po + nr * W_],
                            lhsT, rhs,
                            start=(o_idx == 0), stop=(o_idx == len(OFFS) - 1),
                            tile_position=(32 * i, 32 * j))

                    for o_idx, (dy, dx) in enumerate(OFFS):
                        for i, j in diag:
                            for b in bands:
                                mm(i, j, b, o_idx, dy, dx)
                    # evacuation: vector takes banks 0-1, scalar banks 2-3,
                    # even band first on both engines so its banks free first
                    for b in bands:
                        pv = pst[b].rearrange("p (i f) -> p i f", i=4)
                        nc.vector.tensor_copy(
                            out=otv[:, 0:2, b * NBAND:(b + 1) * NBAND],
                            in_=pv[:, 0:2, 0:NBAND])
                        nc.scalar.copy(
                            out=otv[:, 2:4, b * NBAND:(b + 1) * NBAND],
                            in_=pv[:, 2:4, 0:NBAND])
                    f0 = bands[0] * NBAND
                    f1 = (bands[-1] + 1) * NBAND
                    dst = o_d[n].rearrange("(i p) f -> p i f", p=128)[:, :, f0:f1]
                    # output DMAs go out on the GpSimd and Scalar queues: the
                    # Sync engine then only issues input loads and never
                    # delays them behind output waits
                    nc.gpsimd.dma_start(out=dst[:, 0:2], in_=otv[:, 0:2, f0:f1])
                    nc.scalar.dma_start(out=dst[:, 2:4], in_=otv[:, 2:4, f0:f1])
    _dedupe_ldweights(nc)
    _prune_mm_sem_incs(nc)
    nc.compile()
    return nc


def _prep_weights(W: np.ndarray) -> np.ndarray:
    """W (512, 8, 3, 3) f32 -> (128, 4*9*32) f16 lhsT layout.

    partition p = 32*i + k ; free idx = (j*9 + o)*32 + m
    holds W_blk[4i+j][m, k, dy, dx] for offset o = OFFS[o_idx].
    """
    Wg = W.reshape(64, 8, 8, 3, 3)  # [group gi][oc][ic][dy][dx]
    # block-level dense 32x32 kernels
    Wb = np.zeros((16, 32, 32, 3, 3), dtype=np.float32)  # [b][m(out)][k(in)][dy][dx]
    for gi in range(64):
        b, u = divmod(gi, 4)
        jg = gi + 3 if gi % 4 == 0 else gi - 1  # input group (twist)
        v = jg % 4
        assert jg // 4 == b
        Wb[b, 8 * u:8 * u + 8, 8 * v:8 * v + 8] = Wg[gi]
    out = np.zeros((128, 4 * 9 * 32), dtype=np.float32)
    for i in range(4):
        for j in range(4):
            blk = Wb[4 * i + j]  # [m][k][dy][dx]
            for o_idx, (dy, dx) in enumerate(OFFS):
                # lhsT[k, m]
                out[32 * i:32 * i + 32, (j * 9 + o_idx) * 32:(j * 9 + o_idx) * 32 + 32] = \
                    blk[:, :, dy + 1, dx + 1].T
    return out.astype(np.float16)


def _prep_x(x_shard: np.ndarray) -> np.ndarray:
    """x_shard (BC, 512, 56, 56) f32 -> (BC, 128, 4*HWP) f16 permuted+padded.

    Device partition p = 32*s + k of region r holds original channel
    c = 128*s + 32*r + k (so tile (i,j) reading region j, slice i gets
    block 4i+j), padded to 58 cols.
    """
    n = x_shard.shape[0]
    xs = x_shard.reshape(n, 4, 4, 32, H, W_)          # [n][s][r][k][y][x]
    xs = xs.transpose(0, 1, 3, 2, 4, 5)               # [n][s][k][r][y][x]
    xp = np.zeros((n, 4, 32, 4, H, WP), dtype=np.float16)
    xp[..., 1:57] = xs
    return xp.reshape(n, 128, 4, HWP).reshape(n, 128, 4 * HWP)


def kernel(x: np.ndarray, W: np.ndarray) -> np.ndarray:
    if "nc" not in _CACHE:
        _CACHE["nc"] = _build_nc()
    nc = _CACHE["nc"]

    w_dev = _prep_weights(np.asarray(W, dtype=np.float32))
    x = np.asarray(x, dtype=np.float32)
    in_maps = []
    for c in range(N_CORES):
        shard = x[c * BC:(c + 1) * BC]
        in_maps.append({"x": _prep_x(shard), "w": w_dev})

    res = run_bass_kernel_spmd(nc, in_maps, core_ids=list(range(N_CORES)))
    outs = [res.results[c]["o"].reshape(BC, C, H, W_).astype(np.float32)
            for c in range(N_CORES)]
    return np.concatenate(outs, axis=0)


if __name__ == "__main__":
    # quick self-test against a numpy reference
    rng = np.random.default_rng(0)
    x = rng.standard_normal((B, C, H, W_), dtype=np.float32)
    Wt = (rng.standard_normal((C, 8, 3, 3)) * 0.12).astype(np.float32)
    out = kernel(x, Wt)
    print("out", out.shape, out.dtype)
